# revision 9
# baseline (speedup 1.0000x reference)
"""Trainium2 Bass kernel for nn_DeepSymmetricGCN1dBlock.

3-layer GCN block over a shared 2048-node graph, 32 graph copies (b=4, n=8),
channels 128->256->256->256, per-element branch + symmetric max-pooled branch,
training-mode BatchNorm, ReLU.

Strategy (v4)
-------------
Data-parallel over the 32 graph copies: core k holds copies of graph b=k//2,
n in [4*(k%2), 4*(k%2)+4).  The sparse GCN aggregation is a dense matmul
against the normalized adjacency A_hat [2048, 2048], kept RESIDENT in SBUF
in bf16 (8 MiB), streamed in N=512 moving chunks.  All matmul operands are
bf16 (PSUM accumulation stays fp32); BN statistics are fp32.

Layer 1 runs aggregation-first (agg = x^T A at Cin=128 width; x is uploaded
pre-transposed to node-major by the host).  Layers 2-3 run W-first
(h = x W, then y = h^T A): both input and output stay channel-major, so no
on-device transposes are needed anywhere.

Per layer: 4 element instances first, the (pair-redundant) pooled instance
last.  Collective order per layer on the single CC stream:
  1. element BN-stats AllReduce(add, all 8)  - hidden under pooled matmuls
  2. pooled  BN-stats AllReduce(add, all 8)  - ~13us exposed
  3. pool-max AllReduce(max, core pairs)     - lands during next layer's
                                               element matmuls
BN affine is fused as x' = relu(a1*y1 + (a2*y2 + b1+b2)) with the pooled
term precomputed in place.  GCN biases are skipped (training-mode BN
subtracts the mean, so a per-channel additive constant cancels exactly).
"""

import sys

if "/opt/trn_rl_repo" not in sys.path:
    sys.path.insert(0, "/opt/trn_rl_repo")

import numpy as np
import ml_dtypes

import concourse.bass as bass
import concourse.bacc as bacc
import concourse.mybir as mybir
import concourse.tile as tile
from concourse.bass_utils import run_bass_kernel_spmd

f32 = mybir.dt.float32
bf16 = mybir.dt.bfloat16
AF = mybir.ActivationFunctionType
OP = mybir.AluOpType
AX = mybir.AxisListType

B, N, L, E = 4, 8, 2048, 16384
CH = [128, 256, 256, 256]
EPS = 1e-5
NCORES = 8
GPC = 4            # graph copies per core
LT = L // 128      # 16 node tiles
DQQ = 4            # A streamed in DQQ chunks of DW destination columns
DW = L // DQQ      # 512
CNT_E = 32 * L     # element-branch BN count (all 32 copies)
CNT_P = 8 * L      # pooled-branch BN count (4 graphs, double-counted by pairs)

PAIRS = [[0, 1], [2, 3], [4, 5], [6, 7]]
ALL8 = [list(range(NCORES))]

import os
_PROFILE = False
_SIMULATE = False
_NO_CC = os.environ.get("K_NO_CC", "0") == "1"
_DVE_PSUM = os.environ.get("K_DVE_PSUM", "0") == "1"
_TTR = os.environ.get("K_TTR", "0") == "1"
_CACHE = {}


def _cc(nc, kind, op, groups, bi, bo):
    """bi/bo are APs into DRAM bounce tiles."""
    if _NO_CC:
        nc.sync.dma_start(bo, bi)
    else:
        nc.gpsimd.collective_compute(kind, op, replica_groups=groups,
                                     ins=[bi.opt()], outs=[bo.opt()])


def _emit(tc, nc, io):
    sync, vec, act, te = nc.sync, nc.vector, nc.scalar, nc.tensor

    from contextlib import ExitStack

    ctx = ExitStack()
    with ctx:
        sb = ctx.enter_context(tc.tile_pool(name="sb", bufs=1))
        sb_slot = ctx.enter_context(tc.tile_pool(name="slots", bufs=GPC + 1))
        sb_y1 = ctx.enter_context(tc.tile_pool(name="y1", bufs=GPC + 2))
        sb_h = ctx.enter_context(tc.tile_pool(name="h", bufs=2))
        sb_agg = ctx.enter_context(tc.tile_pool(name="agg", bufs=2))
        sb_w = ctx.enter_context(tc.tile_pool(name="w", bufs=6))
        sb_small = ctx.enter_context(tc.tile_pool(name="small", bufs=28))
        ps_a = ctx.enter_context(tc.tile_pool(name="psa", bufs=2, space="PSUM"))
        ps_w = ctx.enter_context(tc.tile_pool(name="psw", bufs=2, space="PSUM"))
        ps_h = ctx.enter_context(tc.tile_pool(name="psh", bufs=4, space="PSUM"))
        dram = ctx.enter_context(tc.tile_pool(name="dram", bufs=1, space="DRAM"))

        # ---- persistent SBUF tiles -------------------------------------
        Asb = sb.tile([128, DQQ * LT * DW], bf16, tag="Asb")
        slots = [sb_slot.tile([128, LT * 128], bf16, tag="slot", name=f"slot{i}")
                 for i in range(GPC)]
        pool_nm = sb_slot.tile([128, LT * 128], bf16, tag="slot", name="pool_nm")
        y1s = [sb_y1.tile([128, 2 * L], bf16, tag="y1", name=f"y1_{i}")
               for i in range(GPC + 1)]
        poolb = sb_y1.tile([128, 2 * L], bf16, tag="y1", name="poolb")
        mxsb = sb.tile([128, 2 * L], f32, tag="mxsb")
        sqscr = sb.tile([128, L], bf16, tag="sqscr")
        wsb = [sb_w.tile([128, 512], bf16, tag="w", name=f"w{i}") for i in range(6)]
        gbt = sb.tile([128, 24], f32, tag="gbt")
        s1e = sb.tile([128, 2 * GPC * DQQ], f32, tag="s1e")   # [cot][g][dqq]
        s1p = sb.tile([128, 2 * DQQ], f32, tag="s1p")         # [cot][dqq]
        s2e = sb.tile([128, 2 * GPC], f32, tag="s2e")         # [cot][g]
        packe = sb.tile([128, 4], f32, tag="packe")
        warm_sb = sb.tile([128, 1], f32, tag="warm")
        packp = sb.tile([128, 4], f32, tag="packp")
        globe = sb.tile([128, 4], f32, tag="globe")
        globp = sb.tile([128, 4], f32, tag="globp")

        # ---- DRAM bounce tiles for collectives -------------------------
        mx_in = dram.tile([128, 2 * L], f32, tag="mxi")
        mx_out = dram.tile([128, 2 * L], f32, tag="mxo")
        warm_in = dram.tile([128, 1], f32, tag="warmi")
        warm_out = dram.tile([128, 1], f32, tag="warmo")
        ste_in = dram.tile([128, 4], f32, tag="stei")
        ste_out = dram.tile([128, 4], f32, tag="steo")
        stp_in = dram.tile([128, 4], f32, tag="stpi")
        stp_out = dram.tile([128, 4], f32, tag="stpo")

        xsh_d, px_d, ash_d, w_d, gb_d, out_d = (
            io["xsh"], io["pxsh"], io["Ash"], io["Wmats"], io["gbs"], io["out"])

        # ---- input loads ----------------------------------------------
        for g in range(GPC):
            sync.dma_start(slots[g][:], xsh_d[g, :, :])
        sync.dma_start(Asb[:, 0:LT * DW], ash_d[:, 0, :])
        for i in range(6):
            sync.dma_start(wsb[i][:], w_d[i, :, :])
        sync.dma_start(gbt[:], gb_d[:, :])
        for dqq in range(1, DQQ):
            sync.dma_start(Asb[:, dqq * LT * DW:(dqq + 1) * LT * DW],
                           ash_d[:, dqq, :])

        # ---- L1 pooled input: host-computed max over the pair's 8 copies
        sync.dma_start(pool_nm[:], px_d[:, :])

        # warm-up: tiny AllReduce to absorb the comm-init barrier early
        vec.memset(warm_sb[:], 0.0)
        sync.dma_start(warm_in[:], warm_sb[:])
        _cc(nc, "AllReduce", OP.add, ALL8, warm_in[:], warm_out[:])

        # small affine tiles
        eps_t = sb_small.tile([128, 1], f32, tag="sm", name="eps")
        vec.memset(eps_t[:], EPS)
        t0 = sb_small.tile([128, 2], f32, tag="sm", name="t0")
        a1 = sb_small.tile([128, 2], f32, tag="sm", name="a1")
        b1 = sb_small.tile([128, 2], f32, tag="sm", name="b1")
        a2 = sb_small.tile([128, 2], f32, tag="sm", name="a2")
        bs = sb_small.tile([128, 2], f32, tag="bs", name="bs")
        me = sb_small.tile([128, 2], f32, tag="sm", name="me")
        ve = sb_small.tile([128, 2], f32, tag="sm", name="ve")

        def affine(a_t, b_t, s1_ap, s2_ap, inv_cnt, gslc, beslc):
            # a = g * rsqrt(var+eps); b = be - a*mean
            vec.tensor_scalar(me[:], s1_ap, inv_cnt, None, OP.mult)
            vec.tensor_scalar(ve[:], s2_ap, inv_cnt, None, OP.mult)
            vec.tensor_tensor(t0[:], me[:], me[:], OP.mult)
            vec.tensor_tensor(ve[:], ve[:], t0[:], OP.subtract)
            act.activation(t0[:], ve[:], AF.Sqrt, bias=eps_t[:])
            vec.reciprocal(t0[:], t0[:])
            vec.tensor_tensor(a_t[:], gslc, t0[:], OP.mult)
            vec.tensor_tensor(t0[:], a_t[:], me[:], OP.mult)
            vec.tensor_tensor(b_t[:], beslc, t0[:], OP.subtract)

        def element_instance(li, g, src):
            """Emit matmuls + drains for one instance; y lands in y1s[g]."""
            we = wsb[li] if g < GPC else wsb[3 + li]
            dsty = y1s[g] if g < GPC else y1s[GPC]
            s1 = s1e if g < GPC else s1p
            if li == 0:
                # aggregation-first: agg = x^T A (node-major x), then W^T agg
                for dqq in range(DQQ):
                    pa = ps_a.tile([128, DW], f32, tag="psa")
                    for st in range(LT):
                        te.matmul(pa[:], src[:, st * 128:(st + 1) * 128],
                                  Asb[:, (dqq * LT + st) * DW:(dqq * LT + st + 1) * DW],
                                  start=(st == 0), stop=(st == LT - 1))
                    agg = sb_agg.tile([128, DW], bf16, tag="agg")
                    if _DVE_PSUM:
                        vec.tensor_copy(agg[:], pa[:])
                    else:
                        act.activation(agg[:], pa[:], AF.Copy)
                    for cot in range(2):
                        pw = ps_w.tile([128, DW], f32, tag="psw")
                        te.matmul(pw[:], we[:, cot * 128:(cot + 1) * 128], agg[:],
                                  start=True, stop=True)
                        idx = (cot * GPC + g) * DQQ + dqq if g < GPC \
                            else cot * DQQ + dqq
                        act.activation(dsty[:, cot * L + dqq * DW:
                                            cot * L + (dqq + 1) * DW],
                                       pw[:], AF.Copy,
                                       accum_out=s1[:, idx:idx + 1])
            else:
                # W-first: h = x W (node-major h), then y = h^T A
                h = sb_h.tile([128, LT * 256], bf16, tag="h")
                for st in range(LT):
                    ph = ps_h.tile([128, 256], f32, tag="psh")
                    for ct in range(2):
                        te.matmul(ph[:], src[:, ct * L + st * 128:
                                              ct * L + st * 128 + 128],
                                  we[:, ct * 256:(ct + 1) * 256],
                                  start=(ct == 0), stop=(ct == 1))
                    act.activation(h[:, st * 256:(st + 1) * 256], ph[:], AF.Copy)
                for cot in range(2):
                    for dqq in range(DQQ):
                        pw = ps_w.tile([128, DW], f32, tag="psw")
                        for st in range(LT):
                            te.matmul(pw[:],
                                      h[:, st * 256 + cot * 128:
                                        st * 256 + cot * 128 + 128],
                                      Asb[:, (dqq * LT + st) * DW:
                                          (dqq * LT + st + 1) * DW],
                                      start=(st == 0), stop=(st == LT - 1))
                        idx = (cot * GPC + g) * DQQ + dqq if g < GPC \
                            else cot * DQQ + dqq
                        act.activation(dsty[:, cot * L + dqq * DW:
                                            cot * L + (dqq + 1) * DW],
                                       pw[:], AF.Copy,
                                       accum_out=s1[:, idx:idx + 1])

        for li in range(3):
            last = (li == 2)
            # ---- phase 1a: element instances ---------------------------
            for g in range(GPC):
                src = slots[g] if li == 0 else y1s[g]
                element_instance(li, g, src)
                # sum of squares for BN var (bulk, one op per cot)
                for cot in range(2):
                    if _TTR:
                        vec.tensor_tensor_reduce(
                            sqscr[:], y1s[g][:, cot * L:(cot + 1) * L],
                            y1s[g][:, cot * L:(cot + 1) * L], 1.0, 0.0,
                            OP.mult, OP.add,
                            accum_out=s2e[:, cot * GPC + g:cot * GPC + g + 1])
                    else:
                        act.activation(
                            sqscr[:], y1s[g][:, cot * L:(cot + 1) * L],
                            AF.Square,
                            accum_out=s2e[:, cot * GPC + g:cot * GPC + g + 1])
            # element BN-stats AllReduce
            for cot in range(2):
                vec.reduce_sum(packe[:, cot:cot + 1],
                               s1e[:, cot * GPC * DQQ:(cot + 1) * GPC * DQQ], axis=AX.X)
                vec.reduce_sum(packe[:, 2 + cot:3 + cot],
                               s2e[:, cot * GPC:(cot + 1) * GPC], axis=AX.X)
            sync.dma_start(ste_in[:], packe[:])
            _cc(nc, "AllReduce", OP.add, ALL8, ste_in[:], ste_out[:])
            sync.dma_start(globe[:], ste_out[:])

            # ---- phase 1b: pooled instance -----------------------------
            psrc = pool_nm if li == 0 else poolb
            element_instance(li, GPC, psrc)
            for cot in range(2):
                if _TTR:
                    vec.tensor_tensor_reduce(
                        sqscr[:], y1s[GPC][:, cot * L:(cot + 1) * L],
                        y1s[GPC][:, cot * L:(cot + 1) * L], 1.0, 0.0,
                        OP.mult, OP.add, accum_out=packp[:, 2 + cot:3 + cot])
                else:
                    act.activation(
                        sqscr[:], y1s[GPC][:, cot * L:(cot + 1) * L],
                        AF.Square, accum_out=packp[:, 2 + cot:3 + cot])
            for cot in range(2):
                vec.reduce_sum(packp[:, cot:cot + 1],
                               s1p[:, cot * DQQ:(cot + 1) * DQQ], axis=AX.X)
            sync.dma_start(stp_in[:], packp[:])
            _cc(nc, "AllReduce", OP.add, ALL8, stp_in[:], stp_out[:])
            sync.dma_start(globp[:], stp_out[:])

            # ---- phase 2: affines --------------------------------------
            affine(a1, b1, globe[:, 0:2], globe[:, 2:4], 1.0 / CNT_E,
                   gbt[:, 4 * li:4 * li + 2], gbt[:, 4 * li + 2:4 * li + 4])
            affine(a2, bs, globp[:, 0:2], globp[:, 2:4], 1.0 / CNT_P,
                   gbt[:, 12 + 4 * li:14 + 4 * li], gbt[:, 14 + 4 * li:16 + 4 * li])
            vec.tensor_tensor(bs[:], b1[:], bs[:], OP.add)  # b1+b2 combined

            # y2sb = a2*y2 + (b1+b2), in place on the pooled y (ACT engine)
            for cot in range(2):
                act.activation(y1s[GPC][:, cot * L:(cot + 1) * L],
                               y1s[GPC][:, cot * L:(cot + 1) * L],
                               AF.Identity, bias=bs[:, cot:cot + 1],
                               scale=a2[:, cot:cot + 1])

            # ---- phase 3: x' = relu(a1*y1 + y2sb), in place ------------
            for g in range(GPC):
                for cot in range(2):
                    vec.scalar_tensor_tensor(
                        y1s[g][:, cot * L:(cot + 1) * L],
                        y1s[g][:, cot * L:(cot + 1) * L],
                        a1[:, cot:cot + 1],
                        y1s[GPC][:, cot * L:(cot + 1) * L],
                        OP.mult, OP.add)
                    act.activation(y1s[g][:, cot * L:(cot + 1) * L],
                                   y1s[g][:, cot * L:(cot + 1) * L], AF.Relu)
                    if last:
                        sync.dma_start(out_d[g, :, cot * L:(cot + 1) * L],
                                       y1s[g][:, cot * L:(cot + 1) * L])

            if not last:
                # pooled input for the next layer: local max then pair-max
                vec.tensor_max(mxsb[:], y1s[0][:], y1s[1][:])
                vec.tensor_max(mxsb[:], mxsb[:], y1s[2][:])
                vec.tensor_max(mxsb[:], mxsb[:], y1s[3][:])
                sync.dma_start(mx_in[:], mxsb[:])
                _cc(nc, "AllReduce", OP.max, PAIRS, mx_in[:], mx_out[:])
                sync.dma_start(mxsb[:], mx_out[:])
                vec.tensor_copy(poolb[:], mxsb[:])


def _build():
    key = ("nc", _NO_CC, _DVE_PSUM, _TTR)
    if key in _CACHE:
        return _CACHE[key]
    nc = bacc.Bacc("TRN2", target_bir_lowering=False, debug=False,
                   num_devices=NCORES)
    io = {
        "xsh": nc.dram_tensor("xsh", [GPC, 128, LT * 128], bf16,
                              kind="ExternalInput"),
        "pxsh": nc.dram_tensor("pxsh", [128, LT * 128], bf16,
                               kind="ExternalInput"),
        "Ash": nc.dram_tensor("Ash", [128, DQQ, LT * DW], bf16,
                              kind="ExternalInput"),
        "Wmats": nc.dram_tensor("Wmats", [6, 128, 512], bf16,
                                kind="ExternalInput"),
        "gbs": nc.dram_tensor("gbs", [128, 24], f32, kind="ExternalInput"),
        "out": nc.dram_tensor("out", [GPC, 128, 2 * L], bf16,
                              kind="ExternalOutput"),
    }
    with tile.TileContext(nc) as tc:
        _emit(tc, nc, io)
    nc.compile()
    _CACHE[key] = nc
    return nc


def _bf16(a):
    return np.asarray(a, np.float32).astype(ml_dtypes.bfloat16)


def _host_prep(edge_index, Ws, gs, bes):
    """Build the device-layout arrays on host."""
    src = np.asarray(edge_index[0], dtype=np.int64)
    dst = np.asarray(edge_index[1], dtype=np.int64)
    deg = np.zeros(L, np.float32)
    np.add.at(deg, dst, np.float32(1.0))
    deg += np.float32(2.0)
    dis = (1.0 / np.sqrt(deg.astype(np.float64))).astype(np.float32)
    A = np.zeros((L, L), np.float32)
    np.add.at(A, (src, dst), dis[src] * dis[dst])
    A[np.arange(L), np.arange(L)] += np.float32(2.0) * dis * dis
    ash = _bf16(np.ascontiguousarray(
        A.reshape(LT, 128, DQQ, DW).transpose(1, 2, 0, 3).reshape(128, DQQ, LT * DW)))

    wm = np.zeros((6, 128, 512), np.float32)
    for i, W in enumerate(Ws):
        cin = W.shape[0]
        wm[i, :, : (cin // 128) * 256] = np.ascontiguousarray(
            W.reshape(cin // 128, 128, 256).transpose(1, 0, 2).reshape(128, -1))
    wm = _bf16(wm)

    gb = np.zeros((128, 24), np.float32)
    vecs = [gs[0], bes[0], gs[1], bes[1], gs[2], bes[2],
            gs[3], bes[3], gs[4], bes[4], gs[5], bes[5]]
    for v, w in enumerate(vecs):
        gb[:, v * 2 + 0] = w[0:128]
        gb[:, v * 2 + 1] = w[128:256]
    return ash, wm, gb


def kernel(x, edge_index, W1, b1, W2, b2, W3, b3, W1s, b1s, W2s, b2s, W3s, b3s,
           g1, be1, g2, be2, g3, be3, g1s, be1s, g2s, be2s, g3s, be3s):
    x = np.asarray(x, np.float32)
    ash, wm, gb = _host_prep(
        np.asarray(edge_index),
        [np.asarray(W1, np.float32), np.asarray(W2, np.float32),
         np.asarray(W3, np.float32), np.asarray(W1s, np.float32),
         np.asarray(W2s, np.float32), np.asarray(W3s, np.float32)],
        [np.asarray(g1, np.float32), np.asarray(g2, np.float32),
         np.asarray(g3, np.float32), np.asarray(g1s, np.float32),
         np.asarray(g2s, np.float32), np.asarray(g3s, np.float32)],
        [np.asarray(be1, np.float32), np.asarray(be2, np.float32),
         np.asarray(be3, np.float32), np.asarray(be1s, np.float32),
         np.asarray(be2s, np.float32), np.asarray(be3s, np.float32)])

    # core k: graph b=k//2, copies n in [4*(k%2), 4*(k%2)+4)
    # upload x node-major: slot[p, st*128 + c] = x[st*128+p, c]
    xr = x.reshape(NCORES, GPC, CH[0], L)
    xnm = _bf16(np.ascontiguousarray(
        xr.reshape(NCORES, GPC, 128, LT, 128).transpose(0, 1, 4, 3, 2)
          .reshape(NCORES, GPC, 128, LT * 128)))
    # pooled-branch input: max over the 8 copies of each graph, node-major
    xp = x.reshape(B, N, CH[0], L).max(axis=1)  # [B, 128, L]
    xpnm = _bf16(np.ascontiguousarray(
        xp.reshape(B, 128, LT, 128).transpose(0, 3, 2, 1)
          .reshape(B, 128, LT * 128)))
    in_maps = []
    for k in range(NCORES):
        in_maps.append({
            "xsh": xnm[k], "pxsh": xpnm[k // 2],
            "Ash": ash, "Wmats": wm, "gbs": gb,
        })

    nc = _build()

    if _SIMULATE:
        from concourse.bass_interp import MultiCoreSim
        sim = MultiCoreSim(nc, NCORES)
        for k in range(NCORES):
            for nm, arr in in_maps[k].items():
                sim.cores[k].tensor(nm)[:] = arr
        sim.simulate(check_with_hw=False)
        outs = [np.array(sim.cores[k].mem_tensor("out")) for k in range(NCORES)]
        res_outs = outs
    else:
        res = run_bass_kernel_spmd(nc, in_maps, core_ids=list(range(NCORES)),
                                   trace=_PROFILE)
        if _PROFILE:
            _CACHE["last_result"] = res
        res_outs = [np.asarray(res.results[k]["out"]) for k in range(NCORES)]

    # out buf [GPC, 128, 2*L] bf16: y[g, cot*128+p, n] = buf[g, p, cot*L+n]
    full = np.stack([o.astype(np.float32) for o in res_outs])  # [8,G,128,2L]
    full = (full.reshape(NCORES, GPC, 128, 2, L).transpose(0, 1, 3, 2, 4)
                .reshape(NCORES * GPC, 256, L))
    return np.ascontiguousarray(full)
